# revision 50
# baseline (speedup 1.0000x reference)
"""Trainium2 Bass kernel for nn_DiseaseClassifier (segment_reduce).

reference semantics:
    m = mask.astype(f32); counts = m.sum(0)
    pooled = einsum('brh,rd->bdh', x, m) / max(counts,1)
    h = einsum('bdh,dhk->bdk', pooled, W1) + b1
    hn = LN(h) * gamma + beta ; g = gelu_exact(hn)
    preds = einsum('bdk,dk->bd', g, W2) + b2 ; preds[counts==0] = 0

Key algebraic facts used:
  * LayerNorm is scale-invariant, so the 1/count pooling divisor cancels
    (when b1 != 0 we add counts*b1 to the un-normalized pool-matmul output,
    which keeps the invariance exact).
  * b2 rides host-side, and the counts==0 zeroing folds into W2.

Distribution: batch dim sharded over 8 NeuronCores (512 rows each); all
parameters replicated.  Inside each core:
  phase A: pool-matmul.  Stationary = x tiles [(4b,29r)=116, 128h] (bf16),
           moving = 0/1 block-diag mask [116, 56=(14d,4j)] -> PSUM
           [128h, 56], evacuated by ScalarE/VectorE (alternating) into
           pooledT [h, d, b] bf16 layout (double-buffered across chunks so
           phase A of chunk c+1 overlaps phase B of chunk c).
  phase B: per-disease bf16 matmul pooledT[128h,128b] x W1[d][128h,384k]
           (6 h-chunks accumulated in PSUM f32), then bn_stats/bn_aggr ->
           rsqrt -> single ScalarE activation gelu(scale*h+bias) with
           per-partition scale/bias doing the whole LayerNorm, then the
           W2 dot via DVE fused tensor_tensor_reduce (even d) or
           GPSIMD mult + DVE reduce (odd d).  W2 is SBUF-resident
           (loaded once, not per chunk).

Numerics: x, pooled, W1 all bf16 (rel err ~2e-3 measured, gate is 2e-2);
PSUM accumulation, LN stats, gelu, W2 dot all f32.
Tuning notes from measurement: 128-partition fully contiguous DMAs are
4-5x faster than 116-partition/strided; bf16 stationary loads are FWL-
eligible (fp32/f32r are not and serialize the weight load); bulk DMAs
must stay on the sync HWDGE ring (scalar ring shares the busy ACT
sequencer); fp8-lo-plane and dual-ring x DMA both measured slower.
"""

import os
import sys
import functools

for _p in ("/opt/trn_rl_repo", "/opt/pypackages"):
    if os.path.isdir(_p) and _p not in sys.path:
        sys.path.insert(0, _p)

import numpy as np

B, R, H, D = 4096, 29, 768, 14
K = H // 2            # 384
LN_EPS = 1e-5
NCORES = 8
BC = B // NCORES      # 512 batch rows per core
NCHUNK = BC // 128    # 4 chunks of 128 rows
NG = 32               # (4b,29r) groups per chunk
GB = 8                # groups per x-DMA batch (1.57MB per DMA)
HC = H // 128         # 6 contraction chunks
JR = 4 * R            # 116 partitions for the pool matmul
DJ = D * 4            # 56 moving columns of the pool matmul


def _install_walrus_patches():
    """This walrus build supports only ONE sem wait per instruction
    ("Too many sync wait commands").  Split Tile-assigned multi-waits onto
    same-engine NoOps placed right before the instruction, and do the same
    for the TileContext tail drain."""
    from concourse import tile as _tile
    from concourse import mybir
    from concourse.vector_clock import ScopedClock

    if getattr(_tile.TileContext, "_ant_wait_split_patch", False):
        return
    _orig_commit = _tile.TileContext._commit_instruction

    def _patched_commit(self, inst, lazy_reg_writes=True):
        si = getattr(inst, "sync_info", None)
        if si is not None and si.on_wait and len(si.on_wait) > 1:
            waits = list(si.on_wait)
            inst.sync_info = mybir.SyncInfo(
                on_wait=[waits[-1]], on_update=list(si.on_update or [])
            )
            for w in waits[:-1]:
                nop = mybir.InstNoOp(
                    name=self.nc.get_next_instruction_name(), ins=[], outs=[]
                )
                nop.engine = inst.engine
                nop.sync_info = mybir.SyncInfo(on_wait=[w], on_update=[])
                self._add_instruction(nop)
        return _orig_commit(self, inst, lazy_reg_writes)

    def _patched_drain_and_barrier(self, tick_clock, wait_clock):
        drain_inst = self.nc.sync.drain()
        wait_clock.add_sem_waits(
            drain_inst.ins, ScopedClock({None: tick_clock.global_clock})
        )
        si = drain_inst.ins.sync_info
        if si is not None and si.on_wait and len(si.on_wait) > 1:
            waits = list(si.on_wait)
            drain_inst.ins.sync_info = mybir.SyncInfo(
                on_wait=[waits[0]], on_update=list(si.on_update or [])
            )
            for w in waits[1:]:
                d2 = self.nc.sync.drain()
                d2.ins.sync_info = mybir.SyncInfo(on_wait=[w], on_update=[])
        self.nc.all_engine_barrier()
        assert self.sems is not None
        popped = self.nc._tile_sem_poison_stack.pop()
        assert popped is self._sem_poison
        self.nc.clear_and_free_semaphores(list(self.sems.allocated().values()))
        self.nc.all_engine_barrier()

    _tile.TileContext._commit_instruction = _patched_commit
    _tile.TileContext._drain_and_barrier = _patched_drain_and_barrier
    _tile.TileContext._ant_wait_split_patch = True


@functools.lru_cache(maxsize=8)
def build_nc(with_b1: bool = False, with_affine: bool = False, repeat: int = 1, variant: str = "full", SG: int = 4):
    """Build the Bass program (identical on all 8 cores)."""
    import concourse.bass as bass
    import concourse.mybir as mybir
    from concourse.tile import TileContext

    _install_walrus_patches()

    F32 = mybir.dt.float32
    BF16 = mybir.dt.bfloat16
    AF = mybir.ActivationFunctionType
    ALU = mybir.AluOpType

    nc = bass.Bass("TRN2", target_bir_lowering=False, debug=False,
                   num_devices=NCORES)

    x = nc.declare_dram_parameter("x", [NCHUNK, NG // GB, 128, GB * H],
                                  BF16, isOutput=False)
    mblk = nc.declare_dram_parameter("mblk", [JR, DJ], BF16, isOutput=False)
    w1t = nc.declare_dram_parameter("w1t", [128, D, HC, K], BF16, isOutput=False)
    w2rep = nc.declare_dram_parameter("w2rep", [128, D, K], F32, isOutput=False)
    if with_b1:
        b1x = nc.declare_dram_parameter("b1x", [1, D * K], BF16, isOutput=False)
    if with_affine:
        garep = nc.declare_dram_parameter("garep", [128, D, K], F32, isOutput=False)
        berep = nc.declare_dram_parameter("berep", [128, D, K], F32, isOutput=False)
    out = nc.declare_dram_parameter("out", [128, NCHUNK * D], F32, isOutput=True)

    with TileContext(nc) as tc:
        with (
            tc.tile_pool(name="const", bufs=1) as constp,
            tc.tile_pool(name="xin", bufs=3) as xp,
            tc.tile_pool(name="ptp", bufs=2) as ptp,
            tc.tile_pool(name="gly", bufs=2) as gp,
            tc.tile_pool(name="st", bufs=3) as stp,
            tc.tile_pool(name="pg", bufs=4, space="PSUM") as pgp,
            tc.tile_pool(name="hp", bufs=4, space="PSUM") as hpp,
        ):
            mb = constp.tile([JR, DJ], BF16, tag="mblk")
            nc.sync.dma_start(out=mb[:], in_=mblk[:])
            w1sb = constp.tile([128, D, HC, K], BF16, tag="w1sb")
            for d in range(D):
                nc.sync.dma_start(out=w1sb[:, d, :, :], in_=w1t[:, d, :, :])
            w2f32 = constp.tile([128, D, K], F32, tag="w2f32")
            nc.sync.dma_start(
                out=w2f32.rearrange("p d k -> p (d k)"),
                in_=w2rep.rearrange("p d k -> p (d k)"),
            )
            # bf16 copy of W2 for the fused DVE dot (one-time, outside loop)
            w2sb = constp.tile([128, D, K], BF16, tag="w2sb")
            nc.scalar.copy(w2sb[:, :, :], w2f32[:, :, :])

            outsb = constp.tile([128, NCHUNK * D], F32, tag="outsb")
            epst = constp.tile([128, 1], F32, tag="epst")
            nc.vector.memset(epst[:], LN_EPS)
            # constants for the DVE-only quake rsqrt (avoids ACT Sqrt, which
            # would force a 1.28us activation-table reload per Gelu<->Sqrt
            # switch; Copy/Identity/Gelu all share one table)
            I32 = mybir.dt.int32
            sh1 = constp.tile([128, 1], I32, tag="sh1")
            nc.vector.memset(sh1[:], 1)
            cmagic = constp.tile([128, SG], I32, tag="cmagic")
            nc.vector.memset(cmagic[:], 0x5F3759DF)
            nhalf = constp.tile([128, 1], F32, tag="nhalf")
            nc.vector.memset(nhalf[:], -0.5)
            thalf = constp.tile([128, 1], F32, tag="thalf")
            nc.vector.memset(thalf[:], 1.5)
            gtsb = [constp.tile([128, K], BF16, tag=f"gtb{i}", name=f"gtb{i}")
                    for i in range(4)]
            # pooledT slots, manually alternated per chunk (not pool-rotated)
            # so phase B of a body's last chunk can carry into the NEXT body
            # / For_i iteration (the loop barrier orders the carried read).
            pts = [constp.tile([128, HC, D, 128], BF16, tag=f"pt{i}",
                               name=f"pt{i}") for i in range(2)]
            nc.vector.memset(pts[1][:], 0.0)  # first-iteration carry reads this
            tile_idx = [0]
            if variant != "full":
                nc.vector.memset(outsb[:], 0.0)
            if with_b1:
                ones = constp.tile([1, 128], BF16, tag="ones")
                nc.vector.memset(ones[:], 1.0)
                b1sb = constp.tile([1, D * K], BF16, tag="b1sb")
                nc.sync.dma_start(out=b1sb[:], in_=b1x[:])

            def emit_b_mm(pt, d):
                """The 6 accumulating MLP matmuls for one disease."""
                hps = hpp.tile([128, K], F32, tag="hps")
                for hc in range(HC):
                    nc.tensor.matmul(
                        hps[:],
                        lhsT=pt[:, hc, d, :],
                        rhs=w1sb[:, d, hc, :],
                        start=(hc == 0),
                        stop=(hc == HC - 1) and not with_b1,
                    )
                if with_b1:
                    nc.tensor.matmul(
                        hps[:],
                        lhsT=ones[:],
                        rhs=b1sb[:, d * K:(d + 1) * K],
                        start=False,
                        stop=True,
                    )
                return hps

            def emit_b(pt, c, ds, hps_l=None):
                """Phase B tail for diseases `ds` of chunk `c`.  hps_l: already
                -emitted MM outputs (from emit_b_mm interleaving); missing
                entries are emitted here."""
                nsg = len(ds)
                if hps_l is None:
                    hps_l = []
                for i in range(len(hps_l), nsg):
                    hps_l.append(emit_b_mm(pt, ds[i]))
                if variant == "mmonly":
                    return
                agW = stp.tile([128, 2 * nsg], F32, tag="agW")
                sdW = stp.tile([128, nsg], F32, tag="sdW")
                rsW = stp.tile([128, nsg], F32, tag="rsW")
                nmW = stp.tile([128, nsg], F32, tag="nmW")
                for i, d in enumerate(ds):
                    bnst = stp.tile([128, 6], F32, tag="bnst")
                    nc.vector.bn_stats(bnst[:], hps_l[i][:])
                    nc.vector.bn_aggr(agW[:, 2 * i:2 * i + 2], bnst[:])
                if variant == "nostats":
                    return
                # rs = rsqrt(var+eps), entirely on DVE (quake seed + 2 Newton
                # steps, ~1e-5 rel err) so ScalarE never leaves the gelu
                # activation-table set.
                vW = sdW  # reuse tile: v = var + eps
                nc.vector.tensor_scalar(
                    vW[:], agW.rearrange("p (n two) -> p n two", two=2)[:, :, 1],
                    epst[:, 0:1], None, op0=ALU.add,
                )
                t1 = stp.tile([128, nsg], F32, tag="t1")
                t1i = t1[:].bitcast(I32)
                nc.vector.tensor_scalar(
                    t1i, vW[:].bitcast(I32), sh1[:, 0:1], None,
                    op0=ALU.logical_shift_right,
                )
                nc.vector.tensor_tensor(
                    rsW[:].bitcast(I32), cmagic[:, 0:nsg], t1i, op=ALU.subtract,
                )
                for _ in range(2):
                    nc.vector.tensor_tensor(t1[:], vW[:], rsW[:], op=ALU.mult)
                    nc.vector.tensor_tensor(t1[:], t1[:], rsW[:], op=ALU.mult)
                    nc.vector.tensor_scalar(
                        t1[:], t1[:], nhalf[:, 0:1], thalf[:, 0:1],
                        op0=ALU.mult, op1=ALU.add,
                    )
                    nc.vector.tensor_tensor(rsW[:], rsW[:], t1[:], op=ALU.mult)
                # nm = (mu * -1) * rs in one fused DVE op
                nc.vector.scalar_tensor_tensor(
                    nmW[:],
                    agW.rearrange("p (n two) -> p n two", two=2)[:, :, 0],
                    -1.0, rsW[:], op0=ALU.mult, op1=ALU.mult,
                )
                if variant == "nogelu":
                    return
                for i, d in enumerate(ds):
                    gt = gtsb[tile_idx[0] % len(gtsb)]
                    tile_idx[0] += 1
                    if not with_affine:
                        nc.scalar.activation(
                            gt[:], hps_l[i][:], AF.Gelu,
                            bias=nmW[:, i:i + 1], scale=rsW[:, i:i + 1],
                        )
                    else:
                        hn = gp.tile([128, K], F32, tag="hn")
                        gat = gp.tile([128, K], F32, tag="gat")
                        bet = gp.tile([128, K], F32, tag="bet")
                        nc.sync.dma_start(out=gat[:], in_=garep[:, d, :])
                        nc.sync.dma_start(out=bet[:], in_=berep[:, d, :])
                        nc.scalar.activation(
                            hn[:], hps_l[i][:], AF.Identity,
                            bias=nmW[:, i:i + 1], scale=rsW[:, i:i + 1],
                        )
                        nc.vector.tensor_tensor(hn[:], hn[:], gat[:], op=ALU.mult)
                        nc.vector.tensor_tensor(hn[:], hn[:], bet[:], op=ALU.add)
                        nc.scalar.activation(gt[:], hn[:], AF.Gelu)
                    if variant == "nodot":
                        continue
                    # fused DVE dot (bf16 2x): tmpb = gt*w2, accum = sum
                    tmpb = gp.tile([128, K], BF16, tag="tmpb")
                    nc.vector.scalar_tensor_tensor(
                        tmpb[:], gt[:], 1.0, w2sb[:, d, :],
                        op0=ALU.mult, op1=ALU.mult,
                        accum_out=outsb[:, c * D + d:c * D + d + 1],
                    )

            # disease subgroups, one per x-DMA batch slot
            SGS = [list(range(d0, min(d0 + SG, D))) for d0 in range(0, D, SG)]
            assert len(SGS) == NG // GB, "subgroup count must match DMA batches"

            def body(carry_b, emit_tail):
                # Software pipeline: phase B of chunk c-1 is emitted between
                # the x-DMA batches of chunk c, so PE alternates short bursts
                # of pool matmuls (weight-load-bound) and MLP matmuls (moving-
                # bound), and the elementwise tail spreads across the chunk.
                # With carry_b, chunk 0's slots additionally run phase B for
                # the PREVIOUS body's last chunk (pts[1]), removing the
                # barrier-exposed drain.
                prev_pt = pts[1] if carry_b else None
                prev_c = NCHUNK - 1
                for c in range(NCHUNK):
                    pt = pts[c % 2]
                    for gb in range(NG // GB):
                        xt = xp.tile([128, GB * H], BF16, tag="xt")
                        nc.sync.dma_start(out=xt[:], in_=x[c, gb])
                        do_b = prev_pt is not None and variant not in ("dma", "pool")
                        for gg in range(GB):
                            if variant == "dma":
                                continue
                            g = gb * GB + gg
                            pg = pgp.tile([128, HC * DJ], F32, tag="pg")
                            for hc in range(HC):
                                nc.tensor.matmul(
                                    pg[:, hc * DJ:(hc + 1) * DJ],
                                    lhsT=xt[0:JR,
                                            gg * H + hc * 128:gg * H + (hc + 1) * 128],
                                    rhs=mb[:],
                                    start=(hc == 0),
                                    stop=(hc == HC - 1),
                                )
                            # evacuate [128,(hc,d,j)] -> pt[:, hc, d, 4g:4g+4]
                            src = pg.rearrange("p (hc d j) -> p hc d j", hc=HC, d=D)
                            dst = pt[:, :, :, 4 * g:4 * g + 4]
                            if g % 3 == 2:
                                nc.vector.tensor_copy(dst, src)
                            else:
                                nc.scalar.copy(dst, src)
                        if do_b:
                            emit_b(prev_pt, prev_c, SGS[gb])
                    prev_pt, prev_c = pt, c
                if emit_tail and variant not in ("dma", "pool"):
                    for sg in range(len(SGS)):
                        emit_b(prev_pt, NCHUNK - 1, SGS[sg])

            # 2x-unrolled hardware loop: For_i places an all-engine barrier
            # per iteration (sem reset), so each iteration pays full pipeline
            # ramp+drain; two bodies per iteration plus the carry rotation
            # (each body computes the previous body's last-chunk phase B
            # during its own chunk-0 DMA) hide nearly all of that cost.
            # Every repeat computes identical values, so the one-body lag on
            # the last chunk's output leaves the final result unchanged.
            if repeat > 1:
                assert repeat % 2 == 0, "repeat must be even for 2x unroll"
                with tc.For_i(0, repeat // 2, 1):
                    body(carry_b=True, emit_tail=False)
                    body(carry_b=True, emit_tail=False)
            else:
                body(carry_b=False, emit_tail=True)

            nc.sync.dma_start(out=out[:], in_=outsb[:])

    return nc


def _host_prep(region_features, mask, W1, b1, gamma, beta, W2, b2):
    f32 = np.float32
    x = np.ascontiguousarray(region_features, dtype=f32)
    mask = np.asarray(mask)
    counts = mask.astype(np.int64).sum(axis=0)           # [D]
    ind = (counts > 0).astype(f32)                       # [D]

    # block-diag raw 0/1 mask: [(j,r)=116, (d,j)=56]
    import ml_dtypes
    bf16 = ml_dtypes.bfloat16
    mblk = np.zeros((JR, DJ), dtype=bf16)
    mf = mask.astype(f32)                                # [R, D]
    for j in range(4):
        mblk[j * R:(j + 1) * R, :].reshape(R, D, 4)[:, :, j] = mf
    # w1 transposed to [p, d, hc, k] with h = hc*128 + p
    w1t = np.ascontiguousarray(
        np.asarray(W1, dtype=f32).reshape(D, HC, 128, K).transpose(2, 0, 1, 3)
    ).astype(bf16)
    w2eff = np.asarray(W2, dtype=f32) * ind[:, None]
    w2rep = np.ascontiguousarray(
        np.broadcast_to(w2eff[None, :, :], (128, D, K)))
    b2eff = np.asarray(b2, dtype=f32) * ind               # added on host

    b1a = np.asarray(b1, dtype=f32)
    with_b1 = bool(np.any(b1a != 0.0))
    b1x = (b1a * counts.astype(f32)[:, None]).reshape(1, D * K).astype(bf16) if with_b1 else None

    ga = np.asarray(gamma, dtype=f32)
    be = np.asarray(beta, dtype=f32)
    with_affine = bool(np.any(ga != 1.0) or np.any(be != 0.0))
    garep = berep = None
    if with_affine:
        garep = np.ascontiguousarray(np.broadcast_to(ga[None], (128, D, K)))
        berep = np.ascontiguousarray(np.broadcast_to(be[None], (128, D, K)))

    common = {"mblk": mblk, "w1t": w1t, "w2rep": w2rep}
    extra = {"b2eff": b2eff}
    if with_b1:
        common["b1x"] = b1x
    if with_affine:
        common["garep"] = garep
        common["berep"] = berep
    in_maps = []
    for i in range(NCORES):
        m = dict(common)
        # b = c*128 + (gb*GB+gg)*4 + j ; contiguous DMA layout
        xs = x[i * BC:(i + 1) * BC].reshape(NCHUNK, NG // GB, GB, 4, R, H)
        xt_ = xs.transpose(0, 1, 3, 4, 2, 5).reshape(NCHUNK, NG // GB, JR, GB * H)
        xp_ = np.zeros((NCHUNK, NG // GB, 128, GB * H), dtype=bf16)
        xp_[:, :, 0:JR, :] = xt_.astype(bf16)
        m["x"] = xp_
        in_maps.append(m)
    return in_maps, with_b1, with_affine, extra


def kernel(region_features, mask, W1, b1, gamma, beta, W2, b2):
    from concourse.bass_utils import run_bass_kernel_spmd

    in_maps, with_b1, with_affine, extra = _host_prep(
        region_features, mask, W1, b1, gamma, beta, W2, b2
    )
    nc = build_nc(with_b1, with_affine)
    res = run_bass_kernel_spmd(nc, in_maps, list(range(NCORES)))
    outs = []
    for r in res.results:
        o = r["out"].reshape(128, NCHUNK, D).transpose(1, 0, 2).reshape(BC, D)
        outs.append(o)
    full = np.concatenate(outs, axis=0) + extra["b2eff"][None, :]
    return np.ascontiguousarray(full.astype(np.float32))
